# revision 7
# baseline (speedup 1.0000x reference)
"""Trainium2 Bass kernel for degree-3 real spherical-harmonics evaluation.

Computes, for N=2M points with 16 SH coefficients x 2 channels each:
    d    = normalize(coordinates - rx_pos)
    out  = sum_k basis_k(d) * sh[n, k, c]

Strategy (8 NeuronCores, data-parallel over points):
  - Pad N to 2,007,040 = 8 cores * 128 partitions * 1960 points and give each
    core a contiguous slab. Per core, point n lives at (partition p = n//1960,
    f = n%1960); all DMAs are large and fully contiguous per partition.
  - sh coefficients are DMA'd with an inline fp32->bf16 cast (SWDGE), then one
    ScalarE copy de-interleaves them into per-(k,c) planes so the vector MAC
    runs in bf16 2x perf mode with unit stride.
  - The SH basis is built from C1-scaled unit vectors; every SH constant is
    folded into fused DVE ops (scalar_tensor_tensor / affine_mul_reduce /
    tensor_scalar), so no separate scale passes are needed.
  - MAC: 15 broadcasted bf16 multiplies (both channels per instruction) and a
    16-term binary add tree, all in 2x mode.
"""

import numpy as np

import concourse.bass as bass
import concourse.tile as tile
from concourse import bacc, mybir
from concourse.bass_utils import run_bass_kernel_spmd

f32 = mybir.dt.float32
bf16 = mybir.dt.bfloat16
AF = mybir.ActivationFunctionType
OP = mybir.AluOpType

# ----- problem constants (hardcoded per spec) -----
N = 2_000_000
K = 16
CH = 2
ACTIVE_DEG = 3

C0 = 0.28209479177387814
C1 = 0.4886025119029199
C2 = (1.0925484305920792, -1.0925484305920792, 0.31539156525252005,
      -1.0925484305920792, 0.5462742152960396)
C3 = (-0.5900435899266435, 2.890611442640554, -0.4570457994644658,
      0.3731763325901154, -0.4570457994644658, 1.445305721320277,
      -0.5900435899266435)

# Basis constants with the C1 hat-scaling folded in (hats carry a factor C1).
_C12 = C1 * C1
_C13 = C1 * C1 * C1
CC4 = C2[0] / _C12
CC5 = C2[1] / _C12
A6, D6 = 3.0 * C2[2] / _C12, -C2[2]
CC7 = C2[3] / _C12
CC8 = C2[4] / _C12
CC9 = C3[0] / _C13
CC10 = C3[1] / _C13
A11, D11 = 5.0 * C3[2] / _C13, -C3[2] / C1
A12, D12 = 5.0 * C3[3] / _C13, -3.0 * C3[3] / C1
A13, D13 = 5.0 * C3[4] / _C13, -C3[4] / C1
CC14 = C3[5] / _C13
CC15 = C3[6] / _C13

# ----- sharding geometry -----
NCORES = 8
PPART = 1960                 # points per partition per core
PC = 128 * PPART             # points per core = 250,880
NPAD = NCORES * PC           # 2,007,040
TF = 392                     # points per partition per tile
NT = PPART // TF             # 5 tiles


def _build_nc():
    nc = bacc.Bacc("TRN2")
    coords_ext = nc.declare_dram_parameter("coords", [PC, 3], f32, isOutput=False)
    sh_ext = nc.declare_dram_parameter("sh", [PC * K, CH], f32, isOutput=False)
    consts_ext = nc.declare_dram_parameter("consts", [128, 4], f32, isOutput=False)
    out_ext = nc.declare_dram_parameter("out", [PC, CH], f32, isOutput=True)

    coords_ap = coords_ext[:].rearrange("(p f) c -> p (f c)", p=128)   # [128, 5880]
    sh_ap = sh_ext[:].rearrange("(p x) c -> p (x c)", p=128)           # [128, 62720]
    out_ap = out_ext[:].rearrange("(p f) c -> p (f c)", p=128)         # [128, 3920]

    F = TF
    with tile.TileContext(nc) as tc:
        with (
            tc.tile_pool(name="pconst", bufs=1) as pconst,
            tc.tile_pool(name="psh", bufs=2) as psh,
            tc.tile_pool(name="pde", bufs=2) as pde,
            tc.tile_pool(name="pco", bufs=2) as pco,
            tc.tile_pool(name="psq", bufs=2) as psq,
            tc.tile_pool(name="pr", bufs=2) as pr,
            tc.tile_pool(name="ph", bufs=2) as ph,
            tc.tile_pool(name="pmono", bufs=2) as pmono,
            tc.tile_pool(name="pb", bufs=3) as pb,
            tc.tile_pool(name="pm", bufs=4) as pm,
            tc.tile_pool(name="ptree", bufs=8) as ptree,
            tc.tile_pool(name="pacc", bufs=2) as pacc,
            tc.tile_pool(name="pout", bufs=2) as pout,
            tc.tile_pool(name="pscr", bufs=2) as pscr,
        ):
            ct = pconst.tile([128, 4], f32)
            nc.sync.dma_start(out=ct[:], in_=consts_ext[:])

            for t in range(NT):
                shin = psh.tile([128, F * 32], bf16, tag="shin")
                nc.gpsimd.dma_start(
                    out=shin[:], in_=sh_ap[:, t * F * 32:(t + 1) * F * 32]
                )
                ctile = pco.tile([128, F * 3], f32, tag="ctile")
                nc.sync.dma_start(
                    out=ctile[:], in_=coords_ap[:, t * F * 3:(t + 1) * F * 3]
                )

                # de-interleave sh (f,k,c) -> per-(k,c) planes, still bf16
                dein = pde.tile([128, F * 32], bf16, tag="dein")
                nc.scalar.copy(
                    dein[:].rearrange("p (j f) -> p j f", f=F),
                    shin[:].rearrange("p (f j) -> p j f", j=32),
                )

                cv = ctile[:].rearrange("p (f c) -> p c f", c=3)  # strided comps

                # squared offsets (x-rx)^2 via Square's free affine
                sq = psq.tile([128, 3 * F], f32, tag="sq")
                for i in range(3):
                    nc.scalar.activation(
                        sq[:, i * F:(i + 1) * F], cv[:, i, :], AF.Square,
                        bias=ct[:, i:i + 1], scale=1.0,
                    )

                r2a = pr.tile([128, F], f32, tag="r2a")
                nc.vector.tensor_add(r2a[:], sq[:, 0:F], sq[:, F:2 * F])
                r2e = pr.tile([128, F], f32, tag="r2e")
                nc.vector.scalar_tensor_tensor(
                    r2e[:], sq[:, 2 * F:3 * F], 1e-12, r2a[:], OP.add, OP.add
                )
                inv = pr.tile([128, F], f32, tag="inv")
                nc.vector.reciprocal_approx_fast(inv[:], r2e[:])
                # sqrt(C1^2 / r2) = C1 * rsqrt(r2)
                rinv = pr.tile([128, F], f32, tag="rinv")
                nc.scalar.activation(rinv[:], inv[:], AF.Sqrt, bias=0.0,
                                     scale=_C12)

                # C1-scaled unit vector: ((x + (-rx)) * rinvC1), bf16
                hats = ph.tile([128, 3 * F], bf16, tag="hats")
                for i in range(3):
                    nc.vector.scalar_tensor_tensor(
                        hats[:, i * F:(i + 1) * F], cv[:, i, :], ct[:, i:i + 1],
                        rinv[:], OP.add, OP.mult,
                    )
                X = hats[:, 0:F]
                Y = hats[:, F:2 * F]
                Z = hats[:, 2 * F:3 * F]

                sqh = ph.tile([128, 3 * F], bf16, tag="sqh")
                nc.scalar.activation(sqh[:], hats[:], AF.Square, bias=0.0,
                                     scale=1.0)
                XX = sqh[:, 0:F]
                YY = sqh[:, F:2 * F]
                ZZ = sqh[:, 2 * F:3 * F]

                xy = pmono.tile([128, F], bf16, tag="xy")
                nc.vector.tensor_mul(xy[:], X, Y)
                t8 = pmono.tile([128, F], bf16, tag="t8")
                nc.vector.tensor_sub(t8[:], XX, YY)
                u9 = pmono.tile([128, F], bf16, tag="u9")
                nc.vector.affine_then_add(u9[:], XX, t8[:], 2.0, 0.0)
                u15 = pmono.tile([128, F], bf16, tag="u15")
                nc.vector.affine_then_add(u15[:], YY, t8[:], -2.0, 0.0)

                # ---- MAC over k, both channels per instruction ----
                def pair(k):
                    return dein[:, 2 * k * F:(2 * k + 2) * F].rearrange(
                        "p (c f) -> p c f", c=2)

                def bc(apf):
                    return apf.unsqueeze(1).broadcast_to((128, 2, F))

                acc0 = pacc.tile([128, 2 * F], f32, tag="acc0")
                nc.vector.tensor_scalar_mul(acc0[:], dein[:, 0:2 * F], C0)

                def mk_product(k, plane_ap):
                    m = pm.tile([128, 2 * F], bf16, tag="m", name="m")
                    nc.vector.tensor_tensor(
                        m[:].rearrange("p (c f) -> p c f", c=2),
                        bc(plane_ap), pair(k), OP.mult,
                    )
                    return m

                def basis_tile():
                    return pb.tile([128, F], bf16, tag="b", name="b")

                def amr(in0, in1, scale, bias):
                    b = basis_tile()
                    scr = pscr.tile([128, 1], f32, tag="scr", name="scr")
                    nc.vector.affine_mul_reduce(b[:], scr[:], in0, in1, scale, bias)
                    return b

                def tt(a, b_, op):
                    o = ptree.tile([128, 2 * F], f32, tag="tree", name="tr")
                    nc.vector.tensor_tensor(o[:], a[:], b_[:], op)
                    return o

                # Products interleaved with the add tree to keep pool
                # liveness small. Signs: k1, k3 negative; the rest folded
                # into the plane constants.
                m1 = mk_product(1, Y)
                m2 = mk_product(2, Z)
                t1 = tt(m2, m1, OP.subtract)

                m3 = mk_product(3, X)
                b4 = basis_tile()
                nc.vector.tensor_scalar_mul(b4[:], xy[:], CC4)
                m4 = mk_product(4, b4[:])
                t2 = tt(m4, m3, OP.subtract)
                v1 = tt(t1, t2, OP.add)

                b5 = basis_tile()
                nc.vector.scalar_tensor_tensor(b5[:], Y, CC5, Z, OP.mult, OP.mult)
                m5 = mk_product(5, b5[:])
                b6 = basis_tile()
                nc.vector.tensor_scalar(b6[:], ZZ, A6, D6, OP.mult, OP.add)
                m6 = mk_product(6, b6[:])
                t3 = tt(m5, m6, OP.add)

                b7 = basis_tile()
                nc.vector.scalar_tensor_tensor(b7[:], X, CC7, Z, OP.mult, OP.mult)
                m7 = mk_product(7, b7[:])
                b8 = basis_tile()
                nc.vector.tensor_scalar_mul(b8[:], t8[:], CC8)
                m8 = mk_product(8, b8[:])
                t4 = tt(m7, m8, OP.add)
                v2 = tt(t3, t4, OP.add)

                m9 = mk_product(9, amr(u9[:], Y, CC9, 0.0)[:])
                m10 = mk_product(10, amr(xy[:], Z, CC10, 0.0)[:])
                t5 = tt(m9, m10, OP.add)
                m11 = mk_product(11, amr(ZZ, Y, A11, D11)[:])
                m12 = mk_product(12, amr(ZZ, Z, A12, D12)[:])
                t6 = tt(m11, m12, OP.add)
                v3 = tt(t5, t6, OP.add)

                m13 = mk_product(13, amr(ZZ, X, A13, D13)[:])
                m14 = mk_product(14, amr(t8[:], Z, CC14, 0.0)[:])
                t7 = tt(m13, m14, OP.add)
                m15 = mk_product(15, amr(u15[:], X, CC15, 0.0)[:])
                t0 = tt(m15, acc0, OP.add)
                v4 = tt(t7, t0, OP.add)

                w1 = tt(v1, v2, OP.add)
                w2 = tt(v3, v4, OP.add)
                acc = pacc.tile([128, 2 * F], f32, tag="acc")
                nc.vector.tensor_tensor(acc[:], w1[:], w2[:], OP.add)

                # interleave back to (f, c) and upcast to fp32
                out_t = pout.tile([128, 2 * F], f32, tag="out")
                nc.scalar.copy(
                    out_t[:].rearrange("p (f c) -> p c f", c=2),
                    acc[:].rearrange("p (c f) -> p c f", c=2),
                )
                nc.sync.dma_start(
                    out=out_ap[:, t * 2 * F:(t + 1) * 2 * F], in_=out_t[:]
                )

    nc.finalize()
    return nc


_NC_CACHE = None
_last_in_maps = None


def _get_nc():
    global _NC_CACHE
    if _NC_CACHE is None:
        _NC_CACHE = _build_nc()
    return _NC_CACHE


def kernel(coordinates, active_deg, max_coeffs, sh_coefficients, rx_pos,
           **unused):
    assert int(active_deg) == ACTIVE_DEG and int(max_coeffs) == K
    coords = np.ascontiguousarray(np.asarray(coordinates, dtype=np.float32))
    sh = np.ascontiguousarray(np.asarray(sh_coefficients, dtype=np.float32))
    rx = np.asarray(rx_pos, dtype=np.float32).reshape(3)
    n = coords.shape[0]
    assert n == N and sh.shape == (N * K, CH)

    consts = np.zeros((128, 4), dtype=np.float32)
    consts[:, 0:3] = -rx[None, :]

    in_maps = []
    for c in range(NCORES):
        lo, hi = c * PC, (c + 1) * PC
        if hi <= n:
            coords_c = coords[lo:hi]
            sh_c = sh[lo * K:hi * K]
        else:
            real = n - lo
            coords_c = np.zeros((PC, 3), dtype=np.float32)
            coords_c[:real] = coords[lo:]
            sh_c = np.zeros((PC * K, CH), dtype=np.float32)
            sh_c[:real * K] = sh[lo * K:]
        in_maps.append({"coords": coords_c, "sh": sh_c, "consts": consts})

    global _last_in_maps
    _last_in_maps = in_maps
    res = run_bass_kernel_spmd(_get_nc(), in_maps, list(range(NCORES)))
    out = np.concatenate([np.asarray(res.results[c]["out"])
                          for c in range(NCORES)], axis=0)
    return out[:n]
